# revision 7
# baseline (speedup 1.0000x reference)
"""NT-Xent loss (SimCLR) forward on 8 Trainium2 NeuronCores.

Math (faithful to the reference):
    z  = concat(z_i, z_j)                        # [8192, 256]
    zn = z / max(||z||, 1e-8)                    # row-normalize
    S  = (zn @ zn.T) / 0.5                       # [8192, 8192] logits
    labels[i] = i mod 4096
    loss = mean_i( logsumexp_j(S[i, :]) - S[i, label_i] )

Sharding: inputs are replicated to all 8 cores (full I/O), so no collectives
are needed.  Each core c computes the row-block [1024c, 1024c+1024) of S with
an online softmax (no row-max needed: |S| <= 2/T = 4 is bounded since entries
are scaled cosine similarities), and returns the partial sum of the per-row
NLL.  The host sums the 8 partials and divides by 8192.

Per-core kernel design notes:
  * Asymmetric normalization: only the rhs (all 8192 rows) is normalized; the
    lhs uses the raw block rows (bf16, host-cast) and the per-row fp32 1/norm
    rides the ACT engine's per-partition exp() scale together with 1/T.
  * inv = min(exp(-0.5*ln(nsq)), 1e8): Ln/Exp live in one ACT table set
    (forced via the activation-table map) so no table-switch thrash.
  * All transposes go through the DMA xbar (one dma_start_transpose per 1 MB
    chunk), keeping PE free for the 34 GFLOP matmul and PSUM free for two
    4-bank softmax quads; the lhsT tile is transposed straight from DRAM.
  * Norm/scale math runs on bf16 mirrors of the inputs (host-cast) for DVE
    2x modes; label logits are computed separately in fp32.
  * z rows are laid out so every DMA is contiguous per partition; softmax
    column order is a row permutation, which the row-sum doesn't care about.
"""

import functools
import math

import numpy as np

B = 4096
D = 256
NROW = 2 * B  # 8192
NCORES = 8
RPC = NROW // NCORES  # 1024 rows per core
TINV = 2.0  # 1 / temperature
EPS = 1e-8
LN2 = math.log(2.0)

NCH = 8  # chunks of z_full (DMA / transpose granularity)
TPCH = 64 // NCH  # 128-row tiles per chunk

_CACHE = {}


def _patch_act_tables(mybir):
    """Force Exp and Ln to resolve to the combined ACT table set so
    interleaved Ln/Exp never reloads tables (saves ~1.3us per reload)."""
    from concourse import bacc, hw_specs

    if getattr(hw_specs, "_ntx_patched", False):
        return
    orig = hw_specs.get_activation_tables.__wrapped__

    @functools.cache
    def patched(module_arch):
        tables = dict(orig(module_arch))
        comb = "natural_log_exp_and_others"
        FT = mybir.ActivationFunctionType
        if comb in tables:
            for name in tables:
                if name != comb:
                    tables[name] = tables[name] - {FT.Exp, FT.Ln}
        return tables

    hw_specs.get_activation_tables = patched
    bacc.get_activation_tables = patched
    hw_specs._ntx_patched = True


def _build():
    from contextlib import ExitStack

    import concourse.tile as tile
    from concourse import bacc, mybir

    f32 = mybir.dt.float32
    bf16 = mybir.dt.bfloat16
    FT = mybir.ActivationFunctionType
    ALU = mybir.AluOpType
    AX = mybir.AxisListType

    _patch_act_tables(mybir)

    nc = bacc.Bacc("TRN2", target_bir_lowering=False, debug=False)

    z_full_bf = nc.dram_tensor("z_full_bf", [NROW, D], bf16, kind="ExternalInput").ap()
    z_blk = nc.dram_tensor("z_blk", [RPC, D], f32, kind="ExternalInput").ap()
    z_lab = nc.dram_tensor("z_lab", [RPC, D], f32, kind="ExternalInput").ap()
    z_blk_bf = nc.dram_tensor("z_blk_bf", [RPC, D], bf16, kind="ExternalInput").ap()
    out_d = nc.dram_tensor("out_nll", [1, 1], f32, kind="ExternalOutput").ap()

    with tile.TileContext(nc) as tc, ExitStack() as ctx:
        sing = ctx.enter_context(tc.tile_pool(name="sing", bufs=1))
        sq_pool = ctx.enter_context(tc.tile_pool(name="sqp", bufs=4))

        # persistent SBUF tensors
        zin = sing.tile([128, 64 * D], bf16)  # raw z (bf16), row r at [r//64, r%64]
        zn = sing.tile([128, 64 * D], bf16)  # normalized
        znT4 = sing.tile([128, 64, 2, 128], bf16)  # transposed rhs, tile-major
        zblkT = sing.tile([128, 2, RPC], bf16)  # raw block rows, transposed
        normsq = sing.tile([128, 64], f32)
        lnb = sing.tile([128, 64], f32)
        inv_full = sing.tile([128, 64], f32)
        ones1 = sing.tile([128, 1], f32)
        ln2c = sing.tile([128, 1], f32)
        zblk_s = sing.tile([128, 8, D], f32)
        zlab_s = sing.tile([128, 8, D], f32)
        nsq_bl = sing.tile([128, 16], f32)  # cols 0-7: blk, 8-15: lab
        ln_bl = sing.tile([128, 16], f32)
        inv_bl = sing.tile([128, 16], f32)  # 0-7: (1/T)/nrm_blk, 8-15: 1/nrm_lab
        labdot = sing.tile([128, 8], f32)
        slab = sing.tile([128, 8], f32)
        rs = sing.tile([128, 32], f32)  # rowsum partials, col = 4*m + g
        z8 = sing.tile([128, 8], f32)
        nll8 = sing.tile([128, 8], f32)
        nll1 = sing.tile([128, 1], f32)
        out_sb = sing.tile([1, 1], f32)

        nc.vector.memset(ones1, 1.0)
        nc.vector.memset(ln2c, LN2)

        zin3 = zin.rearrange("p (t d) -> p t d", d=D)
        zn3 = zn.rearrange("p (t d) -> p t d", d=D)

        # ---------------- block / label rows ----------------
        nc.sync.dma_start(out=zblk_s, in_=z_blk.rearrange("(p t) d -> p t d", p=128))
        nc.sync.dma_start(out=zlab_s, in_=z_lab.rearrange("(p t) d -> p t d", p=128))
        # lhsT: transpose raw bf16 block rows straight from DRAM via the xbar
        nc.sync.dma_start_transpose(zblkT, z_blk_bf)

        for t in range(8):
            sq_c = sq_pool.tile([128, D], f32, tag="sqf")
            nc.vector.scalar_tensor_tensor(
                out=sq_c, in0=zblk_s[:, t], scalar=1.0, in1=zlab_s[:, t],
                op0=ALU.mult, op1=ALU.mult, accum_out=labdot[:, t : t + 1],
            )
            sq_a = sq_pool.tile([128, D], f32, tag="sqf")
            nc.vector.scalar_tensor_tensor(
                out=sq_a, in0=zblk_s[:, t], scalar=1.0, in1=zblk_s[:, t],
                op0=ALU.mult, op1=ALU.mult, accum_out=nsq_bl[:, t : t + 1],
            )
            sq_b = sq_pool.tile([128, D], f32, tag="sqf")
            nc.vector.scalar_tensor_tensor(
                out=sq_b, in0=zlab_s[:, t], scalar=1.0, in1=zlab_s[:, t],
                op0=ALU.mult, op1=ALU.mult, accum_out=nsq_bl[:, 8 + t : 9 + t],
            )
        nc.scalar.activation(out=ln_bl, in_=nsq_bl, func=FT.Ln)
        nc.scalar.activation(
            out=inv_bl[:, 0:8], in_=ln_bl[:, 0:8], func=FT.Exp, scale=-0.5, bias=ln2c
        )
        nc.scalar.activation(
            out=inv_bl[:, 8:16], in_=ln_bl[:, 8:16], func=FT.Exp, scale=-0.5
        )
        nc.vector.tensor_scalar_min(inv_bl[:, 0:8], inv_bl[:, 0:8], TINV / EPS)
        nc.vector.tensor_scalar_min(inv_bl[:, 8:16], inv_bl[:, 8:16], 1.0 / EPS)
        nc.vector.tensor_tensor(out=slab, in0=labdot, in1=inv_bl[:, 0:8], op=ALU.mult)
        nc.vector.tensor_tensor(out=slab, in0=slab, in1=inv_bl[:, 8:16], op=ALU.mult)

        # ---------- full z: per-chunk load, norm, scale, transpose ----------
        zf = z_full_bf.rearrange("(p c t) d -> p c (t d)", p=128, c=NCH)
        for c in range(NCH):
            csl = slice(2048 * c, 2048 * (c + 1))
            nc.sync.dma_start(out=zin[:, csl], in_=zf[:, c])
            for t in range(TPCH):
                tg = TPCH * c + t
                sq = sq_pool.tile([128, D], bf16, tag="sq")
                nc.vector.scalar_tensor_tensor(
                    out=sq, in0=zin3[:, tg], scalar=1.0, in1=zin3[:, tg],
                    op0=ALU.mult, op1=ALU.mult, accum_out=normsq[:, tg : tg + 1],
                )
            cs = slice(TPCH * c, TPCH * (c + 1))
            nc.scalar.activation(out=lnb[:, cs], in_=normsq[:, cs], func=FT.Ln)
            nc.scalar.activation(
                out=inv_full[:, cs], in_=lnb[:, cs], func=FT.Exp, scale=-0.5
            )
            nc.vector.tensor_scalar_min(inv_full[:, cs], inv_full[:, cs], 1.0 / EPS)
            for t in range(TPCH):
                tg = TPCH * c + t
                if c < 2:
                    # lead-in chunks: scale on ACT (idle before the exps
                    # start) so the first matmul group is ready sooner
                    nc.scalar.activation(
                        out=zn3[:, tg], in_=zin3[:, tg], func=FT.Copy,
                        scale=inv_full[:, tg : tg + 1],
                    )
                else:
                    nc.vector.tensor_scalar(
                        out=zn3[:, tg], in0=zin3[:, tg],
                        scalar1=inv_full[:, tg : tg + 1], scalar2=None, op0=ALU.mult,
                    )
            nc.sync.dma_start_transpose(znT4[:, TPCH * c : TPCH * (c + 1)], zn[:, csl])

        # ---------------- main loop: logits + online softmax ----------------
        with tc.tile_pool(name="qpsum", bufs=2, space="PSUM") as qpsum:
            for g in range(4):  # column group: 2048 cols = tiles 16g..16g+16
                for m in range(8):  # row tile of the block
                    pq = qpsum.tile([128, 2048], f32, tag="q")
                    for h in range(2):
                        for nn in range(4):
                            t0 = 16 * g + 4 * nn
                            nc.tensor.matmul(
                                out=pq[:, 512 * nn : 512 * (nn + 1)],
                                lhsT=zblkT[:, h, 128 * m : 128 * (m + 1)],
                                rhs=znT4[:, t0 : t0 + 4, h, :],
                                start=(h == 0),
                                stop=(h == 1),
                            )
                    nc.scalar.activation(
                        out=pq, in_=pq, func=FT.Exp,
                        scale=inv_bl[:, m : m + 1],
                        accum_out=rs[:, 4 * m + g : 4 * m + g + 1],
                    )

            # ---------------- reduce: nll = log(Z) - s_label ----------------
            nc.vector.tensor_reduce(
                out=z8, in_=rs.rearrange("p (m g) -> p m g", g=4),
                axis=AX.X, op=ALU.add,
            )
            nc.scalar.activation(out=z8, in_=z8, func=FT.Ln)
            nc.vector.tensor_tensor(out=nll8, in0=z8, in1=slab, op=ALU.subtract)
            nc.vector.tensor_reduce(out=nll1, in_=nll8, axis=AX.X, op=ALU.add)
            pfin = qpsum.tile([128, 2048], f32, tag="q")
            nc.tensor.matmul(
                out=pfin[0:1, 0:1], lhsT=ones1, rhs=nll1, start=True, stop=True
            )
            nc.vector.tensor_copy(out=out_sb, in_=pfin[0:1, 0:1])

        nc.sync.dma_start(out=out_d, in_=out_sb)

    nc.compile()
    return nc


def _get_nc():
    if "nc" not in _CACHE:
        _CACHE["nc"] = _build()
    return _CACHE["nc"]


def _make_in_maps(z_i, z_j):
    import ml_dtypes

    z_i = np.ascontiguousarray(np.asarray(z_i, dtype=np.float32))
    z_j = np.ascontiguousarray(np.asarray(z_j, dtype=np.float32))
    z = np.concatenate([z_i, z_j], axis=0)  # [8192, 256]
    z_bf = z.astype(ml_dtypes.bfloat16)

    in_maps = []
    for c in range(NCORES):
        rows = np.arange(c * RPC, (c + 1) * RPC)
        in_maps.append(
            {
                "z_full_bf": z_bf,
                "z_blk": np.ascontiguousarray(z[rows]),
                "z_lab": np.ascontiguousarray(z[rows % B]),
                "z_blk_bf": np.ascontiguousarray(z_bf[rows]),
            }
        )
    return in_maps


def kernel(z_i, z_j, _trace=False):
    from concourse.bass_utils import run_bass_kernel_spmd

    in_maps = _make_in_maps(z_i, z_j)
    nc = _get_nc()
    res = run_bass_kernel_spmd(
        nc, in_maps, core_ids=list(range(NCORES)), trace=_trace
    )
    _CACHE["last_results"] = res
    total = sum(float(res.results[c]["out_nll"][0, 0]) for c in range(NCORES))
    return np.float32(total / NROW)
